# revision 5
# baseline (speedup 1.0000x reference)
"""Bass/Tile TRN2 kernel for nn_DiagonalLSTM — v2 (wide chains + bf16).

Data-parallel over batch: 16 batch elements / 8 cores = 2 per core.

Per core: a 128-step serial LSTM scan over the skewed width dim. The
scan is LATENCY-bound: wall time = 128 x (per-step dependency chain
  ring -> matmul -> sigmoid -> cell(DVE) -> tanh -> ring-write ),
so the design minimizes that chain, not instruction counts:

- TWO chains per core (one per batch element), each 128 rows wide:
  fewer/wider instructions than 4x64 (ACT/DVE fixed costs dominate and
  are width-independent), while still fitting ACT throughput.
- bf16 data end-to-end (tolerance 2e-2): matmuls 1 cycle/row, DVE 2x
  modes, DMA bytes halved.
- Sigmoid AND Tanh live in the same activation table set
  (sigmoid_and_others) -> no table reloads. Using a real Tanh for the
  cell output turns the ring write into a plain TensorTensor mult
  (STT has no DVE perf modes; TT does).
- x is reshaped/padded ON THE HOST to [B*CIN, 255, 128] (j''-major,
  j'' = 254 - (127 + j - i)); the step-t diagonal is then a single
  positive-stride-129 AP, rows i > t landing in a memset zero region.
  The two batch elements sit on partition groups 0:32/32:64 (W2
  duplicated at both bases) which halves per-partition DMA bytes, and
  the DMA is chunked so it hides behind the scan.
- Cell state CH = c/2 (so u = sigmoid(2g)-0.5 needs no extra scaling);
  ring holds full h; tanh ACT computes tanh(2*CH) = tanh(c).
- All four cell ops run back-to-back on DVE (no cross-engine sems on
  the chain); order t1, cgs, u, add.
- Partition-base legality: 2-input DVE ops keep both SBUF inputs at
  equal base partitions: u=(cgs@64, sg_i@64), t1=(sg_f@32, CH@32),
  add=(t1@0, u@0), ring-write=(T2@0, sg_o@0).
"""

import sys

sys.path.insert(0, "/opt/trn_rl_repo")

from contextlib import ExitStack

import numpy as np

import concourse.bass as bass
import concourse.tile as tile
from concourse import bacc, mybir

F32 = mybir.dt.float32
BF16 = mybir.dt.bfloat16
AF = mybir.ActivationFunctionType
ALU = mybir.AluOpType

N_CORES = 8
B = 2  # batch per core
CIN = 32  # input channels
H = 128  # rows (i)
T = 128  # scan steps
BO = 32  # base_out
G4 = 4 * BO  # gate channels (128)
SLOT = B * (H + 1)  # ring slot: [pad, 128 rows] per batch element -> 258
R = 32  # ring depth (slots)
CHUNK = 8  # output DMA chunk, in steps
XSTRIDE = H + 1  # diagonal AP stride in the [q, i] layout (129)


def _build_module(reps=1, t_steps=None, nbufs=2, chunk=CHUNK, stt_u=False):
    TS = T if t_steps is None else t_steps
    nc = bacc.Bacc(
        "TRN2",
        target_bir_lowering=False,
        debug=False,
        num_devices=N_CORES,
    )

    xp_d = nc.dram_tensor("xp", [B * CIN, T, H], BF16, kind="ExternalInput")
    w2t_d = nc.dram_tensor("w2t", [CIN, G4], BF16, kind="ExternalInput")
    w1p_d = nc.dram_tensor("w1p", [BO, G4], BF16, kind="ExternalInput")
    w1c_d = nc.dram_tensor("w1c", [BO, G4], BF16, kind="ExternalInput")
    bias_d = nc.dram_tensor("bias", [G4, 1], F32, kind="ExternalInput")
    scale_d = nc.dram_tensor("scale", [G4, 1], F32, kind="ExternalInput")
    hs_d = nc.dram_tensor("hs", [BO, TS, SLOT], BF16, kind="ExternalOutput")

    with ExitStack() as ctx:
        tc = ctx.enter_context(tile.TileContext(nc))
        const = ctx.enter_context(tc.tile_pool(name="const", bufs=1))
        psum = ctx.enter_context(tc.tile_pool(name="psum", bufs=2, space="PSUM"))
        sig_p = ctx.enter_context(tc.tile_pool(name="sig", bufs=nbufs))
        tmp_p = ctx.enter_context(tc.tile_pool(name="tmp", bufs=nbufs))

        # ---- persistent tiles ----
        xs = const.tile([B * CIN, T * H], BF16, tag="xs")
        zq = const.tile([B * CIN, H], BF16, tag="zq")  # zero matmul rhs
        ring = const.tile([BO, R * SLOT], BF16, tag="ring")
        chbig = {
            b: const.tile([2 * BO, H], BF16, tag=f"ch{b}", name=f"chbig{b}")
            for b in range(B)
        }
        zb = const.tile([2 * BO, 1], F32, tag="zb")  # zero bias @ base 32
        w2tb = const.tile([B * CIN, G4], BF16, tag="w2tb")  # dup at base 0/32
        w1p = const.tile([BO, G4], BF16, tag="w1p")
        w1c = const.tile([BO, G4], BF16, tag="w1c")
        bias = const.tile([G4, 1], F32, tag="bias")
        scale = const.tile([G4, 1], F32, tag="scale")

        # ---- preamble: tiny memsets + DMAs (no compute) ----
        nc.vector.memset(zq[:, :], 0.0)
        # ring: only the pad columns (q=0 per b) are read before written
        rpad = ring[:, :].rearrange("p (s b q) -> p s b q", s=R, b=B)[
            :, :, :, 0:1
        ]
        nc.vector.memset(rpad, 0.0)
        for b in range(B):
            nc.vector.memset(chbig[b][:, :], 0.0)
        nc.vector.memset(zb[:, :], 0.0)
        # x data region, chunked high-q first (step t reads
        # q in [127 - t, 127], so a small first chunk unblocks step 0
        # fastest); weight DMAs interleave after it.
        xs3 = xs[:, :].rearrange("p (j i) -> p j i", i=H)
        nc.sync.dma_start(
            out=xs3[:, 120:128, :], in_=xp_d.ap()[:, 120:128, :]
        )
        for b in range(B):
            nc.sync.dma_start(
                out=w2tb[32 * b : 32 * b + 32, :], in_=w2t_d.ap()
            )
        nc.sync.dma_start(out=bias[:, :], in_=bias_d.ap())
        nc.sync.dma_start(out=scale[:, :], in_=scale_d.ap())
        nc.sync.dma_start(out=w1p[:, :], in_=w1p_d.ap())
        nc.sync.dma_start(out=w1c[:, :], in_=w1c_d.ap())
        for j0, j1 in ((96, 120), (64, 96), (32, 64), (0, 32)):
            nc.sync.dma_start(
                out=xs3[:, j0:j1, :], in_=xp_d.ap()[:, j0:j1, :]
            )

        rv = ring[:, :].rearrange("p (s b q) -> p s b q", s=R, b=B)

        import contextlib

        rep_ctx = tc.For_i(0, reps, 1) if reps > 1 else contextlib.nullcontext()
        with rep_ctx:
            for t in range(TS):
                sp = (t - 1) % R
                sl = t % R
                has_state = t > 0

                # input term: one diagonal (stride 129) matmul per
                # chain over the valid rows i <= t; a zero-matmul
                # initializes the remaining PSUM columns (skewed x = 0
                # there), so no zero region is stored in SBUF.
                s0 = H * (T - 1 - t)
                nv = t + 1
                gs = {}
                for b in range(B):
                    g = psum.tile([G4, H], F32, tag=f"g{b}", name=f"g{b}")
                    gs[b] = g
                    full = nv >= H
                    if not full:
                        nc.tensor.matmul(
                            g[:, :],
                            w2tb[32 * b : 32 * b + 32, :],
                            zq[32 * b : 32 * b + 32, :],
                            start=True,
                            stop=False,
                        )
                    nc.tensor.matmul(
                        g[:, 0:nv],
                        w2tb[32 * b : 32 * b + 32, :],
                        xs[
                            32 * b : 32 * b + 32,
                            s0 : s0 + XSTRIDE * (nv - 1) + 1 : XSTRIDE,
                        ],
                        start=full,
                        stop=not has_state,
                    )
                if has_state:
                    for b in range(B):
                        nc.tensor.matmul(
                            gs[b][:, :],
                            w1p[:, :],
                            rv[:, sp, b, 0:H],
                            start=False,
                            stop=False,
                        )
                        nc.tensor.matmul(
                            gs[b][:, :],
                            w1c[:, :],
                            rv[:, sp, b, 1 : 1 + H],
                            start=False,
                            stop=True,
                        )

                sgs = {}
                for b in range(B):
                    sg = sig_p.tile([G4, H], BF16, tag=f"sg{b}", name=f"sg{b}")
                    nc.scalar.activation(
                        sg[:, :], gs[b][:, :], AF.Sigmoid, bias=bias[:, :],
                        scale=scale[:, :],
                    )
                    sgs[b] = sg

                for b in range(B):
                    sg = sgs[b]
                    cgb = tmp_p.tile([3 * BO, H], BF16, tag=f"cgb{b}",
                                     name=f"cgb{b}")
                    cgs = cgb[2 * BO : 3 * BO, :]
                    u = tmp_p.tile([BO, H], BF16, tag=f"u{b}", name=f"u{b}")
                    t1 = tmp_p.tile([BO, H], BF16, tag=f"t1{b}", name=f"t1{b}")
                    t2 = tmp_p.tile([BO, H], BF16, tag=f"t2{b}", name=f"t2{b}")
                    ch = chbig[b][BO : 2 * BO, :]
                    nc.vector.tensor_tensor(
                        t1[:, :], sg[BO : 2 * BO, :], ch, ALU.mult
                    )
                    if stt_u:
                        nc.vector.scalar_tensor_tensor(
                            u[:, :], sg[3 * BO : 4 * BO, :], 0.5,
                            sg[2 * BO : 3 * BO, :], ALU.subtract, ALU.mult,
                        )
                    else:
                        nc.vector.tensor_scalar_sub(
                            cgs, sg[3 * BO : 4 * BO, :], 0.5
                        )
                        nc.vector.tensor_tensor(
                            u[:, :], cgs, sg[2 * BO : 3 * BO, :], ALU.mult
                        )
                    nc.vector.tensor_tensor(ch, t1[:, :], u[:, :], ALU.add)
                    nc.scalar.activation(
                        t2[:, :], ch, AF.Tanh, bias=zb[BO : 2 * BO, :],
                        scale=2.0,
                    )
                    nc.vector.tensor_tensor(
                        rv[:, sl, b, 1 : 1 + H], t2[:, :], sg[0:BO, :],
                        ALU.mult,
                    )

                if t % chunk == chunk - 1:
                    c0 = t - chunk + 1
                    s0c = c0 % R
                    nc.sync.dma_start(
                        out=hs_d.ap()[:, c0 : t + 1, :],
                        in_=ring[:, s0c * SLOT : (s0c + chunk) * SLOT],
                    )

    nc.compile()
    return nc


_NC_CACHE = {}


def _get_module(**kw):
    key = tuple(sorted(kw.items()))
    if key not in _NC_CACHE:
        _NC_CACHE[key] = _build_module(**kw)
    return _NC_CACHE[key]


def _prep_host_inputs(x, W2, b2, W1, b1):
    """Host-side preprocessing shared by all cores (weights) + layouted x."""
    import ml_dtypes

    bf16 = ml_dtypes.bfloat16
    x = np.asarray(x, np.float32)
    W2 = np.asarray(W2, np.float32)
    W1 = np.asarray(W1, np.float32)
    b1 = np.asarray(b1, np.float32)
    b2 = np.asarray(b2, np.float32)

    w2t = np.ascontiguousarray(W2.T).astype(bf16)  # (CIN, G4)
    w1p = np.ascontiguousarray(W1[:, :, 0].T).astype(bf16)  # (BO, G4)
    w1c = np.ascontiguousarray(W1[:, :, 1].T).astype(bf16)
    bias = (b1 + b2).astype(np.float32)
    bias[3 * BO :] *= 2.0
    bias = np.ascontiguousarray(bias[:, None])
    scale = np.ones((G4, 1), np.float32)
    scale[3 * BO :] = 2.0

    # x layout [b*CIN, q, i]: xp[bc, q, i] = x[b, c, i, 127 - q]; the
    # step-t diagonal (row i reads q = 127 - t + i, valid rows only) is
    # a single stride-129 AP.
    nb = x.shape[0]
    xr = x[:, :, :, ::-1]  # (B, C, H, T), reversed j
    xp = np.ascontiguousarray(xr.transpose(0, 1, 3, 2))
    xp = xp.reshape(nb * CIN, T, H).astype(bf16)
    return xp, w2t, w1p, w1c, bias, scale


def make_in_maps(x, W2, b2, W1, b1):
    xp, w2t, w1p, w1c, bias, scale = _prep_host_inputs(x, W2, b2, W1, b1)
    return [
        {
            "xp": np.ascontiguousarray(
                xp[B * CIN * k : B * CIN * (k + 1)]
            ),
            "w2t": w2t,
            "w1p": w1p,
            "w1c": w1c,
            "bias": bias,
            "scale": scale,
        }
        for k in range(N_CORES)
    ]


def kernel(x, W2, b2, W1, b1):
    from concourse.bass_utils import run_bass_kernel_spmd

    nc = _get_module()
    in_maps = make_in_maps(x, W2, b2, W1, b1)
    res = run_bass_kernel_spmd(nc, in_maps, list(range(N_CORES)))
    out = np.empty((N_CORES * B, BO, H, T), np.float32)
    for k in range(N_CORES):
        hs = np.asarray(res.results[k]["hs"]).astype(np.float32)
        v = hs.reshape(BO, T, B, H + 1)[:, :, :, 1:]  # (o, t, b, i)
        out[B * k : B * k + B] = v.transpose(2, 0, 3, 1)
    return out


# revision 6
# speedup vs baseline: 1.0404x; 1.0404x over previous
"""Bass/Tile TRN2 kernel for nn_DiagonalLSTM — v2 (wide chains + bf16).

Data-parallel over batch: 16 batch elements / 8 cores = 2 per core.

Per core: a 128-step serial LSTM scan over the skewed width dim. The
scan is LATENCY-bound: wall time = 128 x (per-step dependency chain
  ring -> matmul -> sigmoid -> cell(DVE) -> tanh -> ring-write ),
so the design minimizes that chain, not instruction counts:

- TWO chains per core (one per batch element), each 128 rows wide:
  fewer/wider instructions than 4x64 (ACT/DVE fixed costs dominate and
  are width-independent), while still fitting ACT throughput.
- bf16 data end-to-end (tolerance 2e-2): matmuls 1 cycle/row, DVE 2x
  modes, DMA bytes halved.
- Sigmoid AND Tanh live in the same activation table set
  (sigmoid_and_others) -> no table reloads. Using a real Tanh for the
  cell output turns the ring write into a plain TensorTensor mult
  (STT has no DVE perf modes; TT does).
- x is reshaped ON THE HOST to [B*CIN, 128, 128] (q-major, q = 127-j);
  the step-t input term is ONE stride-129 diagonal AP over the valid
  rows i <= t, plus a zero-matmul initializing the remaining PSUM
  columns (skewed x = 0 there). The two batch elements sit on
  partition groups 0:32/32:64 (W2 duplicated at both bases) which
  halves per-partition DMA bytes; the DMA is chunked high-q-first
  (small first chunk) so the scan starts ~3us in and the rest hides
  behind it.
- Cell state CH = c/2 (so u = sigmoid(2g)-0.5 needs no extra scaling);
  ring holds full h; tanh ACT computes tanh(2*CH) = tanh(c).
- All four cell ops run back-to-back on DVE (no cross-engine sems on
  the chain); order t1, cgs, u, add.
- Partition-base legality: 2-input DVE ops keep both SBUF inputs at
  equal base partitions: u=(cgs@64, sg_i@64), t1=(sg_f@32, CH@32),
  add=(t1@0, u@0), ring-write=(T2@0, sg_o@0).
"""

import sys

sys.path.insert(0, "/opt/trn_rl_repo")

from contextlib import ExitStack

import numpy as np

import concourse.bass as bass
import concourse.tile as tile
from concourse import bacc, mybir

F32 = mybir.dt.float32
BF16 = mybir.dt.bfloat16
AF = mybir.ActivationFunctionType
ALU = mybir.AluOpType

N_CORES = 8
B = 2  # batch per core
CIN = 32  # input channels
H = 128  # rows (i)
T = 128  # scan steps
BO = 32  # base_out
G4 = 4 * BO  # gate channels (128)
SLOT = B * (H + 1)  # ring slot: [pad, 128 rows] per batch element -> 258
R = 32  # ring depth (slots)
CHUNK = 8  # output DMA chunk, in steps
XSTRIDE = H + 1  # diagonal AP stride in the [q, i] layout (129)


def _build_module(reps=1, t_steps=None, nbufs=2, chunk=CHUNK, stt_u=False):
    TS = T if t_steps is None else t_steps
    nc = bacc.Bacc(
        "TRN2",
        target_bir_lowering=False,
        debug=False,
        num_devices=N_CORES,
    )

    xp_d = nc.dram_tensor("xp", [B * CIN, T, H], BF16, kind="ExternalInput")
    w2t_d = nc.dram_tensor("w2t", [CIN, G4], BF16, kind="ExternalInput")
    w1p_d = nc.dram_tensor("w1p", [BO, G4], BF16, kind="ExternalInput")
    w1c_d = nc.dram_tensor("w1c", [BO, G4], BF16, kind="ExternalInput")
    bias_d = nc.dram_tensor("bias", [G4, 1], F32, kind="ExternalInput")
    scale_d = nc.dram_tensor("scale", [G4, 1], F32, kind="ExternalInput")
    hs_d = nc.dram_tensor("hs", [BO, TS, SLOT], BF16, kind="ExternalOutput")

    with ExitStack() as ctx:
        tc = ctx.enter_context(tile.TileContext(nc))
        const = ctx.enter_context(tc.tile_pool(name="const", bufs=1))
        psum = ctx.enter_context(tc.tile_pool(name="psum", bufs=2, space="PSUM"))
        sig_p = ctx.enter_context(tc.tile_pool(name="sig", bufs=nbufs))
        tmp_p = ctx.enter_context(tc.tile_pool(name="tmp", bufs=nbufs))

        # ---- persistent tiles ----
        xs = const.tile([B * CIN, T * H], BF16, tag="xs")
        zq = const.tile([B * CIN, H], BF16, tag="zq")  # zero matmul rhs
        ring = const.tile([BO, R * SLOT], BF16, tag="ring")
        chbig = {
            b: const.tile([2 * BO, H], BF16, tag=f"ch{b}", name=f"chbig{b}")
            for b in range(B)
        }
        zb = const.tile([2 * BO, 1], F32, tag="zb")  # zero bias @ base 32
        w2tb = const.tile([B * CIN, G4], BF16, tag="w2tb")  # dup at base 0/32
        w1p = const.tile([BO, G4], BF16, tag="w1p")
        w1c = const.tile([BO, G4], BF16, tag="w1c")
        bias = const.tile([G4, 1], F32, tag="bias")
        scale = const.tile([G4, 1], F32, tag="scale")

        # ---- preamble: tiny memsets + DMAs (no compute) ----
        nc.vector.memset(zq[:, :], 0.0)
        # ring: only the pad columns (q=0 per b) are read before written
        rpad = ring[:, :].rearrange("p (s b q) -> p s b q", s=R, b=B)[
            :, :, :, 0:1
        ]
        nc.vector.memset(rpad, 0.0)
        for b in range(B):
            nc.vector.memset(chbig[b][:, :], 0.0)
        nc.vector.memset(zb[:, :], 0.0)
        # x data region, chunked high-q first (step t reads
        # q in [127 - t, 127], so a small first chunk unblocks step 0
        # fastest); weight DMAs interleave after it.
        xs3 = xs[:, :].rearrange("p (j i) -> p j i", i=H)
        nc.sync.dma_start(
            out=xs3[:, 120:128, :], in_=xp_d.ap()[:, 120:128, :]
        )
        for b in range(B):
            nc.sync.dma_start(
                out=w2tb[32 * b : 32 * b + 32, :], in_=w2t_d.ap()
            )
        nc.sync.dma_start(out=bias[:, :], in_=bias_d.ap())
        nc.sync.dma_start(out=scale[:, :], in_=scale_d.ap())
        nc.sync.dma_start(out=w1p[:, :], in_=w1p_d.ap())
        nc.sync.dma_start(out=w1c[:, :], in_=w1c_d.ap())
        for j0, j1 in ((96, 120), (64, 96), (32, 64), (0, 32)):
            nc.sync.dma_start(
                out=xs3[:, j0:j1, :], in_=xp_d.ap()[:, j0:j1, :]
            )

        rv = ring[:, :].rearrange("p (s b q) -> p s b q", s=R, b=B)

        import contextlib

        rep_ctx = tc.For_i(0, reps, 1) if reps > 1 else contextlib.nullcontext()
        with rep_ctx:
            for t in range(TS):
                sp = (t - 1) % R
                sl = t % R
                has_state = t > 0

                # input term: one diagonal (stride 129) matmul per
                # chain over the valid rows i <= t; a zero-matmul
                # initializes the remaining PSUM columns (skewed x = 0
                # there), so no zero region is stored in SBUF.
                s0 = H * (T - 1 - t)
                nv = t + 1
                gs = {}
                for b in range(B):
                    g = psum.tile([G4, H], F32, tag=f"g{b}", name=f"g{b}")
                    gs[b] = g
                    full = nv >= H
                    if not full:
                        nc.tensor.matmul(
                            g[:, :],
                            w2tb[32 * b : 32 * b + 32, :],
                            zq[32 * b : 32 * b + 32, :],
                            start=True,
                            stop=False,
                        )
                    nc.tensor.matmul(
                        g[:, 0:nv],
                        w2tb[32 * b : 32 * b + 32, :],
                        xs[
                            32 * b : 32 * b + 32,
                            s0 : s0 + XSTRIDE * (nv - 1) + 1 : XSTRIDE,
                        ],
                        start=full,
                        stop=not has_state,
                    )
                if has_state:
                    for b in range(B):
                        nc.tensor.matmul(
                            gs[b][:, :],
                            w1p[:, :],
                            rv[:, sp, b, 0:H],
                            start=False,
                            stop=False,
                        )
                        nc.tensor.matmul(
                            gs[b][:, :],
                            w1c[:, :],
                            rv[:, sp, b, 1 : 1 + H],
                            start=False,
                            stop=True,
                        )

                sgs = {}
                for b in range(B):
                    sg = sig_p.tile([G4, H], BF16, tag=f"sg{b}", name=f"sg{b}")
                    nc.scalar.activation(
                        sg[:, :], gs[b][:, :], AF.Sigmoid, bias=bias[:, :],
                        scale=scale[:, :],
                    )
                    sgs[b] = sg

                for b in range(B):
                    sg = sgs[b]
                    cgb = tmp_p.tile([3 * BO, H], BF16, tag=f"cgb{b}",
                                     name=f"cgb{b}")
                    cgs = cgb[2 * BO : 3 * BO, :]
                    u = tmp_p.tile([BO, H], BF16, tag=f"u{b}", name=f"u{b}")
                    t1 = tmp_p.tile([BO, H], BF16, tag=f"t1{b}", name=f"t1{b}")
                    t2 = tmp_p.tile([BO, H], BF16, tag=f"t2{b}", name=f"t2{b}")
                    ch = chbig[b][BO : 2 * BO, :]
                    nc.vector.tensor_tensor(
                        t1[:, :], sg[BO : 2 * BO, :], ch, ALU.mult
                    )
                    if stt_u:
                        nc.vector.scalar_tensor_tensor(
                            u[:, :], sg[3 * BO : 4 * BO, :], 0.5,
                            sg[2 * BO : 3 * BO, :], ALU.subtract, ALU.mult,
                        )
                    else:
                        nc.vector.tensor_scalar_sub(
                            cgs, sg[3 * BO : 4 * BO, :], 0.5
                        )
                        nc.vector.tensor_tensor(
                            u[:, :], cgs, sg[2 * BO : 3 * BO, :], ALU.mult
                        )
                    nc.vector.tensor_tensor(ch, t1[:, :], u[:, :], ALU.add)
                    nc.scalar.activation(
                        t2[:, :], ch, AF.Tanh, bias=zb[BO : 2 * BO, :],
                        scale=2.0,
                    )
                    nc.vector.tensor_tensor(
                        rv[:, sl, b, 1 : 1 + H], t2[:, :], sg[0:BO, :],
                        ALU.mult,
                    )

                if t % chunk == chunk - 1:
                    c0 = t - chunk + 1
                    s0c = c0 % R
                    nc.sync.dma_start(
                        out=hs_d.ap()[:, c0 : t + 1, :],
                        in_=ring[:, s0c * SLOT : (s0c + chunk) * SLOT],
                    )

    nc.compile()
    return nc


_NC_CACHE = {}


def _get_module(**kw):
    key = tuple(sorted(kw.items()))
    if key not in _NC_CACHE:
        _NC_CACHE[key] = _build_module(**kw)
    return _NC_CACHE[key]


def _prep_host_inputs(x, W2, b2, W1, b1):
    """Host-side preprocessing shared by all cores (weights) + layouted x."""
    import ml_dtypes

    bf16 = ml_dtypes.bfloat16
    x = np.asarray(x, np.float32)
    W2 = np.asarray(W2, np.float32)
    W1 = np.asarray(W1, np.float32)
    b1 = np.asarray(b1, np.float32)
    b2 = np.asarray(b2, np.float32)

    w2t = np.ascontiguousarray(W2.T).astype(bf16)  # (CIN, G4)
    w1p = np.ascontiguousarray(W1[:, :, 0].T).astype(bf16)  # (BO, G4)
    w1c = np.ascontiguousarray(W1[:, :, 1].T).astype(bf16)
    bias = (b1 + b2).astype(np.float32)
    bias[3 * BO :] *= 2.0
    bias = np.ascontiguousarray(bias[:, None])
    scale = np.ones((G4, 1), np.float32)
    scale[3 * BO :] = 2.0

    # x layout [b*CIN, q, i]: xp[bc, q, i] = x[b, c, i, 127 - q]; the
    # step-t diagonal (row i reads q = 127 - t + i, valid rows only) is
    # a single stride-129 AP.
    nb = x.shape[0]
    xr = x[:, :, :, ::-1]  # (B, C, H, T), reversed j
    xp = np.ascontiguousarray(xr.transpose(0, 1, 3, 2))
    xp = xp.reshape(nb * CIN, T, H).astype(bf16)
    return xp, w2t, w1p, w1c, bias, scale


def make_in_maps(x, W2, b2, W1, b1):
    xp, w2t, w1p, w1c, bias, scale = _prep_host_inputs(x, W2, b2, W1, b1)
    return [
        {
            "xp": np.ascontiguousarray(
                xp[B * CIN * k : B * CIN * (k + 1)]
            ),
            "w2t": w2t,
            "w1p": w1p,
            "w1c": w1c,
            "bias": bias,
            "scale": scale,
        }
        for k in range(N_CORES)
    ]


def kernel(x, W2, b2, W1, b1):
    from concourse.bass_utils import run_bass_kernel_spmd

    nc = _get_module()
    in_maps = make_in_maps(x, W2, b2, W1, b1)
    res = run_bass_kernel_spmd(nc, in_maps, list(range(N_CORES)))
    out = np.empty((N_CORES * B, BO, H, T), np.float32)
    for k in range(N_CORES):
        hs = np.asarray(res.results[k]["hs"]).astype(np.float32)
        v = hs.reshape(BO, T, B, H + 1)[:, :, :, 1:]  # (o, t, b, i)
        out[B * k : B * k + B] = v.transpose(2, 0, 3, 1)
    return out


# revision 7
# speedup vs baseline: 1.0425x; 1.0020x over previous
"""Bass/Tile TRN2 kernel for nn_DiagonalLSTM — v2 (wide chains + bf16).

Data-parallel over batch: 16 batch elements / 8 cores = 2 per core.

Per core: a 128-step serial LSTM scan over the skewed width dim. The
scan is LATENCY-bound: wall time = 128 x (per-step dependency chain
  ring -> matmul -> sigmoid -> cell(DVE) -> tanh -> ring-write ),
so the design minimizes that chain, not instruction counts:

- TWO chains per core (one per batch element), each 128 rows wide:
  fewer/wider instructions than 4x64 (ACT/DVE fixed costs dominate and
  are width-independent), while still fitting ACT throughput.
- bf16 data end-to-end (tolerance 2e-2): matmuls 1 cycle/row, DVE 2x
  modes, DMA bytes halved.
- Sigmoid AND Tanh live in the same activation table set
  (sigmoid_and_others) -> no table reloads. Using a real Tanh for the
  cell output turns the ring write into a plain TensorTensor mult
  (STT has no DVE perf modes; TT does).
- x is reshaped ON THE HOST to [B*CIN, 128, 128] (q-major, q = 127-j);
  the step-t input term is ONE stride-129 diagonal AP over the valid
  rows i <= t, plus a zero-matmul initializing the remaining PSUM
  columns (skewed x = 0 there). The two batch elements sit on
  partition groups 0:32/32:64 (W2 duplicated at both bases) which
  halves per-partition DMA bytes; the DMA is chunked high-q-first
  (small first chunk) so the scan starts ~3us in and the rest hides
  behind it.
- Cell state CH = c/2 (so u = sigmoid(2g)-0.5 needs no extra scaling);
  ring holds full h; tanh ACT computes tanh(2*CH) = tanh(c).
- All four cell ops run back-to-back on DVE (no cross-engine sems on
  the chain); order t1, cgs, u, add.
- Partition-base legality: 2-input DVE ops keep both SBUF inputs at
  equal base partitions: u=(cgs@64, sg_i@64), t1=(sg_f@32, CH@32),
  add=(t1@0, u@0), ring-write=(T2@0, sg_o@0).
"""

import sys

sys.path.insert(0, "/opt/trn_rl_repo")

from contextlib import ExitStack

import numpy as np

import concourse.bass as bass
import concourse.tile as tile
from concourse import bacc, mybir

F32 = mybir.dt.float32
BF16 = mybir.dt.bfloat16
AF = mybir.ActivationFunctionType
ALU = mybir.AluOpType

N_CORES = 8
B = 2  # batch per core
CIN = 32  # input channels
H = 128  # rows (i)
T = 128  # scan steps
BO = 32  # base_out
G4 = 4 * BO  # gate channels (128)
SLOT = B * (H + 1)  # ring slot: [pad, 128 rows] per batch element -> 258
R = 32  # ring depth (slots)
CHUNK = 4  # output DMA chunk, in steps
XSTRIDE = H + 1  # diagonal AP stride in the [q, i] layout (129)


def _build_module(reps=1, t_steps=None, nbufs=2, chunk=CHUNK, stt_u=False):
    TS = T if t_steps is None else t_steps
    nc = bacc.Bacc(
        "TRN2",
        target_bir_lowering=False,
        debug=False,
        num_devices=N_CORES,
    )

    xp_d = nc.dram_tensor("xp", [B * CIN, T, H], BF16, kind="ExternalInput")
    w2t_d = nc.dram_tensor("w2t", [CIN, G4], BF16, kind="ExternalInput")
    w1p_d = nc.dram_tensor("w1p", [BO, G4], BF16, kind="ExternalInput")
    w1c_d = nc.dram_tensor("w1c", [BO, G4], BF16, kind="ExternalInput")
    bias_d = nc.dram_tensor("bias", [G4, 1], F32, kind="ExternalInput")
    scale_d = nc.dram_tensor("scale", [G4, 1], F32, kind="ExternalInput")
    hs_d = nc.dram_tensor("hs", [BO, TS, SLOT], BF16, kind="ExternalOutput")

    with ExitStack() as ctx:
        tc = ctx.enter_context(tile.TileContext(nc))
        const = ctx.enter_context(tc.tile_pool(name="const", bufs=1))
        psum = ctx.enter_context(tc.tile_pool(name="psum", bufs=2, space="PSUM"))
        sig_p = ctx.enter_context(tc.tile_pool(name="sig", bufs=nbufs))
        tmp_p = ctx.enter_context(tc.tile_pool(name="tmp", bufs=nbufs))

        # ---- persistent tiles ----
        xs = const.tile([B * CIN, T * H], BF16, tag="xs")
        zq = const.tile([B * CIN, H], BF16, tag="zq")  # zero matmul rhs
        ring = const.tile([BO, R * SLOT], BF16, tag="ring")
        chbig = {
            b: const.tile([2 * BO, H], BF16, tag=f"ch{b}", name=f"chbig{b}")
            for b in range(B)
        }
        zb = const.tile([2 * BO, 1], F32, tag="zb")  # zero bias @ base 32
        w2tb = const.tile([B * CIN, G4], BF16, tag="w2tb")  # dup at base 0/32
        w1p = const.tile([BO, G4], BF16, tag="w1p")
        w1c = const.tile([BO, G4], BF16, tag="w1c")
        bias = const.tile([G4, 1], F32, tag="bias")
        scale = const.tile([G4, 1], F32, tag="scale")

        # ---- preamble: tiny memsets + DMAs (no compute) ----
        nc.vector.memset(zq[:, :], 0.0)
        # ring: only the pad columns (q=0 per b) are read before written
        rpad = ring[:, :].rearrange("p (s b q) -> p s b q", s=R, b=B)[
            :, :, :, 0:1
        ]
        nc.vector.memset(rpad, 0.0)
        for b in range(B):
            nc.vector.memset(chbig[b][:, :], 0.0)
        nc.vector.memset(zb[:, :], 0.0)
        # x data region, chunked high-q first (step t reads
        # q in [127 - t, 127], so a small first chunk unblocks step 0
        # fastest); weight DMAs interleave after it.
        xs3 = xs[:, :].rearrange("p (j i) -> p j i", i=H)
        nc.sync.dma_start(
            out=xs3[:, 120:128, :], in_=xp_d.ap()[:, 120:128, :]
        )
        for b in range(B):
            nc.sync.dma_start(
                out=w2tb[32 * b : 32 * b + 32, :], in_=w2t_d.ap()
            )
        nc.sync.dma_start(out=bias[:, :], in_=bias_d.ap())
        nc.sync.dma_start(out=scale[:, :], in_=scale_d.ap())
        nc.sync.dma_start(out=w1p[:, :], in_=w1p_d.ap())
        nc.sync.dma_start(out=w1c[:, :], in_=w1c_d.ap())
        for j0, j1 in ((96, 120), (64, 96), (32, 64), (0, 32)):
            nc.sync.dma_start(
                out=xs3[:, j0:j1, :], in_=xp_d.ap()[:, j0:j1, :]
            )

        rv = ring[:, :].rearrange("p (s b q) -> p s b q", s=R, b=B)

        import contextlib

        rep_ctx = tc.For_i(0, reps, 1) if reps > 1 else contextlib.nullcontext()
        with rep_ctx:
            for t in range(TS):
                sp = (t - 1) % R
                sl = t % R
                has_state = t > 0

                # input term: one diagonal (stride 129) matmul per
                # chain over the valid rows i <= t; a zero-matmul
                # initializes the remaining PSUM columns (skewed x = 0
                # there), so no zero region is stored in SBUF.
                s0 = H * (T - 1 - t)
                nv = t + 1
                gs = {}
                for b in range(B):
                    g = psum.tile([G4, H], F32, tag=f"g{b}", name=f"g{b}")
                    gs[b] = g
                    full = nv >= H
                    if not full:
                        nc.tensor.matmul(
                            g[:, :],
                            w2tb[32 * b : 32 * b + 32, :],
                            zq[32 * b : 32 * b + 32, :],
                            start=True,
                            stop=False,
                        )
                    nc.tensor.matmul(
                        g[:, 0:nv],
                        w2tb[32 * b : 32 * b + 32, :],
                        xs[
                            32 * b : 32 * b + 32,
                            s0 : s0 + XSTRIDE * (nv - 1) + 1 : XSTRIDE,
                        ],
                        start=full,
                        stop=not has_state,
                    )
                if has_state:
                    for b in range(B):
                        nc.tensor.matmul(
                            gs[b][:, :],
                            w1p[:, :],
                            rv[:, sp, b, 0:H],
                            start=False,
                            stop=False,
                        )
                        nc.tensor.matmul(
                            gs[b][:, :],
                            w1c[:, :],
                            rv[:, sp, b, 1 : 1 + H],
                            start=False,
                            stop=True,
                        )

                sgs = {}
                for b in range(B):
                    sg = sig_p.tile([G4, H], BF16, tag=f"sg{b}", name=f"sg{b}")
                    nc.scalar.activation(
                        sg[:, :], gs[b][:, :], AF.Sigmoid, bias=bias[:, :],
                        scale=scale[:, :],
                    )
                    sgs[b] = sg

                for b in range(B):
                    sg = sgs[b]
                    cgb = tmp_p.tile([3 * BO, H], BF16, tag=f"cgb{b}",
                                     name=f"cgb{b}")
                    cgs = cgb[2 * BO : 3 * BO, :]
                    u = tmp_p.tile([BO, H], BF16, tag=f"u{b}", name=f"u{b}")
                    t1 = tmp_p.tile([BO, H], BF16, tag=f"t1{b}", name=f"t1{b}")
                    t2 = tmp_p.tile([BO, H], BF16, tag=f"t2{b}", name=f"t2{b}")
                    ch = chbig[b][BO : 2 * BO, :]
                    nc.vector.tensor_tensor(
                        t1[:, :], sg[BO : 2 * BO, :], ch, ALU.mult
                    )
                    if stt_u:
                        nc.vector.scalar_tensor_tensor(
                            u[:, :], sg[3 * BO : 4 * BO, :], 0.5,
                            sg[2 * BO : 3 * BO, :], ALU.subtract, ALU.mult,
                        )
                    else:
                        nc.vector.tensor_scalar_sub(
                            cgs, sg[3 * BO : 4 * BO, :], 0.5
                        )
                        nc.vector.tensor_tensor(
                            u[:, :], cgs, sg[2 * BO : 3 * BO, :], ALU.mult
                        )
                    nc.vector.tensor_tensor(ch, t1[:, :], u[:, :], ALU.add)
                    nc.scalar.activation(
                        t2[:, :], ch, AF.Tanh, bias=zb[BO : 2 * BO, :],
                        scale=2.0,
                    )
                    nc.vector.tensor_tensor(
                        rv[:, sl, b, 1 : 1 + H], t2[:, :], sg[0:BO, :],
                        ALU.mult,
                    )

                if t % chunk == chunk - 1:
                    c0 = t - chunk + 1
                    s0c = c0 % R
                    nc.sync.dma_start(
                        out=hs_d.ap()[:, c0 : t + 1, :],
                        in_=ring[:, s0c * SLOT : (s0c + chunk) * SLOT],
                    )

    nc.compile()
    return nc


_NC_CACHE = {}


def _get_module(**kw):
    key = tuple(sorted(kw.items()))
    if key not in _NC_CACHE:
        _NC_CACHE[key] = _build_module(**kw)
    return _NC_CACHE[key]


def _prep_host_inputs(x, W2, b2, W1, b1):
    """Host-side preprocessing shared by all cores (weights) + layouted x."""
    import ml_dtypes

    bf16 = ml_dtypes.bfloat16
    x = np.asarray(x, np.float32)
    W2 = np.asarray(W2, np.float32)
    W1 = np.asarray(W1, np.float32)
    b1 = np.asarray(b1, np.float32)
    b2 = np.asarray(b2, np.float32)

    w2t = np.ascontiguousarray(W2.T).astype(bf16)  # (CIN, G4)
    w1p = np.ascontiguousarray(W1[:, :, 0].T).astype(bf16)  # (BO, G4)
    w1c = np.ascontiguousarray(W1[:, :, 1].T).astype(bf16)
    bias = (b1 + b2).astype(np.float32)
    bias[3 * BO :] *= 2.0
    bias = np.ascontiguousarray(bias[:, None])
    scale = np.ones((G4, 1), np.float32)
    scale[3 * BO :] = 2.0

    # x layout [b*CIN, q, i]: xp[bc, q, i] = x[b, c, i, 127 - q]; the
    # step-t diagonal (row i reads q = 127 - t + i, valid rows only) is
    # a single stride-129 AP.
    nb = x.shape[0]
    xr = x[:, :, :, ::-1]  # (B, C, H, T), reversed j
    xp = np.ascontiguousarray(xr.transpose(0, 1, 3, 2))
    xp = xp.reshape(nb * CIN, T, H).astype(bf16)
    return xp, w2t, w1p, w1c, bias, scale


def make_in_maps(x, W2, b2, W1, b1):
    xp, w2t, w1p, w1c, bias, scale = _prep_host_inputs(x, W2, b2, W1, b1)
    return [
        {
            "xp": np.ascontiguousarray(
                xp[B * CIN * k : B * CIN * (k + 1)]
            ),
            "w2t": w2t,
            "w1p": w1p,
            "w1c": w1c,
            "bias": bias,
            "scale": scale,
        }
        for k in range(N_CORES)
    ]


def kernel(x, W2, b2, W1, b1):
    from concourse.bass_utils import run_bass_kernel_spmd

    nc = _get_module()
    in_maps = make_in_maps(x, W2, b2, W1, b1)
    res = run_bass_kernel_spmd(nc, in_maps, list(range(N_CORES)))
    out = np.empty((N_CORES * B, BO, H, T), np.float32)
    for k in range(N_CORES):
        hs = np.asarray(res.results[k]["hs"]).astype(np.float32)
        v = hs.reshape(BO, T, B, H + 1)[:, :, :, 1:]  # (o, t, b, i)
        out[B * k : B * k + B] = v.transpose(2, 0, 3, 1)
    return out
